# revision 56
# baseline (speedup 1.0000x reference)
"""MinGRU layer Trainium2 kernel.

Math (per batch b):
    g = x @ Wg + bg ; v = x @ Wv + bv ; d = x @ Wd + bd
    xs = sigmoid(g) * tanh(v) ; a = 0.001 + 0.998 * sigmoid(d)
    h_t = a_t * h_{t-1} + xs_t  (h_0 = 0, scan over time S)

Sharding: 8 cores = 4 batches x 2 halves of the 1024 output features.
Each core computes h^T[e, s] for its (b, e-half) with zero cross-core
communication; the time recurrence runs on-chip via the VectorE
TensorTensorScan instruction (time on the free axis, features on
partitions; scan state is fp32 internally regardless of operand dtype).

Precision: matmul inputs (x, W) and all post-activation intermediates
are fp16 (PSUM accumulation and the scan state stay fp32) — measured
end-to-end rel err ~4e-4 vs the 2e-2 budget, and it halves DMA traffic
and SBUF footprint. Host feeds x transposed (d-major) so every device
access is contiguous.

Schedule per superchunk u (= chunk pair 2u, 2u+1):
  PE: warmup matmuls at t=0 ramp the HAM clock gate; then for p(3
      proj): for j(4 e-blocks): k-loop with the two chunks' matmuls
      interleaved on banks (same-bank back-to-back accumulation is
      slow), sharing each weight tile.
  ACT: sig(g) and tanh(v) per (j) over both chunks; sig(d) per (t, j)
      so the tail drains at (t, j) granularity.
  DVE: per (t, j): gating multiply, decay affine a = 0.998*sig(d) +
      0.001, then the scan with carry chaining.
  SP: weights/bias at startup, then one store per (chunk, j).
  POOL/SCALAR: input DMAs on two independent rings at startup.
"""

import os
import sys

for _p in ("/opt/trn_rl_repo", "/root/.axon_site/_ro/trn_rl_repo"):
    if os.path.isdir(_p) and _p not in sys.path:
        sys.path.insert(0, _p)

import numpy as np

import concourse.bass as bass
import concourse.mybir as mybir
from concourse import bass_utils

B, S, D = 4, 4096, 1024
E = 512                # output features per core (D / 2)
NCH = 8                # time chunks
SC = S // NCH          # chunk length (512)
KT = D // 128          # contraction tiles (8)
JB = E // 128          # output-feature blocks per core (4)

F32 = mybir.dt.float32
F16 = mybir.dt.float16
AF = mybir.ActivationFunctionType
OP = mybir.AluOpType

N_WARMUP = 20          # 128-col PE matmuls bridging t=0 to the first real
                       # matmul (~2.8us) so the clock-gate ramp starts early


def _build_bass(nch=NCH, mode="full"):
    """Build the Bass program. nch > NCH replays the 8 data chunks multiple
    times (benchmarking only — amortizes host/RPC overhead out of timing).
    mode="pe" keeps only PE + input DMAs (bottleneck isolation)."""
    assert nch % 2 == 0
    nc = bass.Bass("TRN2", target_bir_lowering=False, debug=False, num_devices=8)

    xt_d = nc.dram_tensor("xt", [D, S], F16, kind="ExternalInput").ap()
    w_d = nc.dram_tensor("w", [3, D, E], F16, kind="ExternalInput").ap()
    bias_d = nc.dram_tensor("bias", [128, 3 * JB], F32, kind="ExternalInput").ap()
    ht_d = nc.dram_tensor("ht", [E, S], F16, kind="ExternalOutput").ap()

    from contextlib import ExitStack

    with ExitStack() as ctx:
        block = ctx.enter_context(nc.Block())
        sem_xt = ctx.enter_context(nc.semaphore("sem_xt"))
        sem_xtA = ctx.enter_context(nc.semaphore("sem_xtA"))
        sem_xtB = ctx.enter_context(nc.semaphore("sem_xtB"))
        sem_xt2 = ctx.enter_context(nc.semaphore("sem_xt2"))
        sem_xt2A = ctx.enter_context(nc.semaphore("sem_xt2A"))
        sem_xt2B = ctx.enter_context(nc.semaphore("sem_xt2B"))
        sem_w = ctx.enter_context(nc.semaphore("sem_w"))
        sem_wA = ctx.enter_context(nc.semaphore("sem_wA"))
        sem_wB = ctx.enter_context(nc.semaphore("sem_wB"))
        sem_b = ctx.enter_context(nc.semaphore("sem_b"))
        sem_warm = ctx.enter_context(nc.semaphore("sem_warm"))
        sem_pe = ctx.enter_context(nc.semaphore("sem_pe"))
        sem_act = ctx.enter_context(nc.semaphore("sem_act"))
        sem_dve = ctx.enter_context(nc.semaphore("sem_dve"))
        # stores alternate between two sems so consecutive stores never
        # chain-wait on each other's completion (the ~0.9us DMA-sem
        # propagation would otherwise sit on the drain's critical path)
        sem_st = ctx.enter_context(nc.semaphore("sem_st"))
        sem_st2 = ctx.enter_context(nc.semaphore("sem_st2"))
        w_sb = ctx.enter_context(nc.sbuf_tensor("w_sb", [128, 3, KT, E], F16))
        # two pair-slots: each holds a superchunk (2 chunks side by side on
        # the free axis) so steady-state loads are one DMA with 2KB runs
        xt_sb = ctx.enter_context(
            nc.sbuf_tensor("xt_sb", [128, 2, KT, 2 * SC], F16)
        )
        bias_sb = ctx.enter_context(nc.sbuf_tensor("bias_sb", [128, 3 * JB], F32))
        warm_sb = ctx.enter_context(nc.sbuf_tensor("warm_sb", [128, 128], F16))
        # leading dim: superchunk parity (double buffer) — without it the
        # ACT(u) ops chain on DVE(u-1) ops which chain on ACT(u-1), aligning
        # the whole consumer pipeline just-in-time behind the PE and costing
        # the PE ~426ns at every (p, j) block boundary
        sig_g = ctx.enter_context(nc.sbuf_tensor("sig_g", [128, 2, 2, JB, SC], F16))
        tanh_v = ctx.enter_context(nc.sbuf_tensor("tanh_v", [128, 2, 2, JB, SC], F16))
        sig_d = ctx.enter_context(nc.sbuf_tensor("sig_d", [128, 2, 2, JB, SC], F16))
        a_t = ctx.enter_context(nc.sbuf_tensor("a_t", [128, 2, JB, SC], F16))
        xs_t = ctx.enter_context(nc.sbuf_tensor("xs_t", [128, 2, JB, SC], F16))
        h_t = ctx.enter_context(nc.sbuf_tensor("h_t", [128, 2, JB, SC], F16))
        ps = []
        for j in range(JB):
            ps_j = ctx.enter_context(nc.psum_tensor(f"ps{j}", [128, 2, SC], F32))
            ps.append(ps_j)

        # x^T viewed as [p, k, s]; row index of xt is d = 128*k + p
        xt_view = xt_d.rearrange("(k p) s -> p k s", p=128)
        # weights viewed as [p, proj, k, e]
        w_view = w_d.rearrange("q (k p) e -> p q k e", p=128)
        # h^T viewed as [p, j, s]; row index of ht is e = 128*j + p
        ht_view = ht_d.rearrange("(j p) s -> p j s", p=128)

        nsc = nch // 2

        # PE group counter: groups complete in (u, p, j, t) order
        def grp_done(u, p, j, t):
            return 24 * u + 8 * p + 2 * j + t + 1

        # ACT op counter per superchunk: g(j0..3), v(j0..3), d(t0,j0..3),
        # d(t1,j0..3) = 16 ops
        def act_g(u, j):
            return 16 * u + 1 + j

        def act_v(u, j):
            return 16 * u + 5 + j

        def act_d(u, t, j):
            # d ops run (j outer, t inner) so each j's pair completes right
            # after its PE p2 group — keeps the drain j-granular
            return 16 * u + 9 + 2 * j + t

        # DVE op counter per superchunk: per j: per t: mult, aff, scan
        # (j outer so only j3's groups drain after the PE's final p2 group;
        # per-j scan carry chains stay in order since t0 precedes t1)
        def dve_mult(u, t, j):
            return 24 * u + 6 * j + 3 * t + 1

        def dve_aff(u, t, j):
            return 24 * u + 6 * j + 3 * t + 2

        def dve_scan(u, t, j):
            return 24 * u + 6 * j + 3 * t + 3

        # store counter: (u, j, t) order matching scan completion order.
        # Store #p (1-based) rides sem_st if p is odd, sem_st2 if even, and
        # is that sem's ((p+1)//2)-th increment.
        def st_pos(c, j):
            return 8 * (c // 2) + 2 * j + (c % 2) + 1

        def st_sem(p):
            return sem_st if p % 2 == 1 else sem_st2

        def st_val(p):
            return 16 * ((p + 1) // 2)

        @block.gpsimd
        def _(gpsimd):
            # Cumulative-sem soundness: SDMA engine-slots drain independently,
            # so a threshold 16*n on a sem is only sound when ALL DMAs queued
            # on that sem at that point are covered by it. Hence separate
            # sems per stream; later loads are queue-gated on sem_pe so every
            # downstream wait is a full-prefix wait.
            # Chunk 0 rides here (SWDGE); chunk 1 rides the scalar HWDGE ring
            # in parallel. k-halves let PE start on the low contraction tiles.
            # each startup piece rides its own semaphore: two unguarded DMAs
            # on one sem drain their 16 slot-increments interleaved, so a
            # partial threshold would be unsound. Quarters keep the PE's
            # time-to-first-matmul at ~2.6us (k0-1 arrive first).
            gpsimd.dma_start(
                xt_sb[:, 0, 0:2, 0:SC], xt_view[:, 0:2, 0:SC]
            ).then_inc(sem_xtA, 16)
            gpsimd.dma_start(
                xt_sb[:, 0, 2:4, 0:SC], xt_view[:, 2:4, 0:SC]
            ).then_inc(sem_xtB, 16)
            gpsimd.dma_start(
                xt_sb[:, 0, KT // 2 :, 0:SC], xt_view[:, KT // 2 :, 0:SC]
            ).then_inc(sem_xt, 16)
            for up in range(1, nch // 2):
                # pair up's slot (up%2) was last used by pair up-2, consumed
                # by the end of superchunk up-2 — a full superchunk of
                # prefetch lead. The sem_xt chain wait keeps this sem's
                # increments strictly sequential (DMA slot-completions
                # interleave otherwise). sem_xt counts: c0-hi=16, pair up at
                # 16*(up+1).
                gpsimd.wait_ge(sem_xt, 16 * up)
                if up == 1:
                    # throttle off the startup-critical first microseconds
                    gpsimd.wait_ge(sem_pe, 2)
                else:
                    gpsimd.wait_ge(sem_pe, grp_done(up - 2, 2, 3, 1))
                s_lo = SC * ((2 * up) % NCH)
                gpsimd.dma_start(
                    xt_sb[:, up % 2, :, :],
                    xt_view[:, :, s_lo : s_lo + 2 * SC],
                ).then_inc(sem_xt, 16)

        @block.tensor
        def _(tensor):
            # Warmup: tiny matmuls on a DVE-memset SBUF tile ramp the PE
            # HAM clock gate toward full speed while the first DMAs stream
            # in; their psum garbage is overwritten by the first real
            # start=True group.
            if N_WARMUP:
                tensor.wait_ge(sem_warm, 1)
                for _ in range(N_WARMUP):
                    tensor.matmul(
                        ps[0][0:8, 0, 0:128], warm_sb[:, 0:8], warm_sb[:, :],
                        start=True, stop=True,
                    )
            for u in range(nsc):
                if u >= 1:
                    # this pair resident (pair u lands at 16*(u+1))
                    tensor.wait_ge(sem_xt, 16 * (u + 1))
                sl = u % 2
                for p in range(3):
                    if u == 0 and p >= 1:
                        # this projection's weights resident (p=0 is gated
                        # k-granularly inside the first j-loop below)
                        tensor.wait_ge(sem_w, (32, 48)[p - 1])
                    for j in range(JB):
                        # banks (2j, 2j+1) were written by the previous
                        # p-block; the first matmul of this block carries a
                        # wait for the ACT ops that read them (attached, not
                        # a standalone EventSemaphore — a standalone wait
                        # breaks the PE pipeline and costs ~426ns/block)
                        if (u, p) == (0, 0) or mode == "pe":
                            blk_wait = None
                        elif p == 0:
                            blk_wait = act_d(u - 1, 1, j)
                        elif p == 1:
                            blk_wait = act_g(u, j)
                        else:
                            blk_wait = act_v(u, j)
                        for k in range(KT):
                            if u == 0 and p == 0 and j == 0:
                                if k == 0:
                                    # first quarter (k0-1) of both startup
                                    # chunks and of the Wg weights
                                    tensor.wait_ge(sem_xtA, 16)
                                    tensor.wait_ge(sem_xt2A, 16)
                                    tensor.wait_ge(sem_wA, 16)
                                elif k == 2:
                                    tensor.wait_ge(sem_xtB, 16)
                                    tensor.wait_ge(sem_xt2B, 16)
                                    tensor.wait_ge(sem_wB, 16)
                                elif k == KT // 2:
                                    tensor.wait_ge(sem_xt, 16)
                                    tensor.wait_ge(sem_xt2, 16)
                                    tensor.wait_ge(sem_w, 16)
                            w_ap = w_sb[:, p, k, 128 * j : 128 * (j + 1)]
                            m0 = tensor.matmul(
                                ps[j][:, 0, :], w_ap,
                                xt_sb[:, sl, k, 0:SC],
                                start=(k == 0), stop=(k == KT - 1),
                            )
                            if k == 0 and blk_wait is not None:
                                m0._wait_ge(sem_act, blk_wait)
                            m1 = tensor.matmul(
                                ps[j][:, 1, :], w_ap,
                                xt_sb[:, sl, k, SC : 2 * SC],
                                start=(k == 0), stop=(k == KT - 1),
                            )
                        # per-t incs: odd sem_pe values mean "t0 group done"
                        # (one matmul earlier); even values land exactly when
                        # the old +2 did, so all even thresholds are unchanged
                        m0.then_inc(sem_pe, 1)
                        m1.then_inc(sem_pe, 1)

        @block.scalar
        def _(scalar):
            # Startup: chunk 1 loads ride the otherwise-idle ACT HWDGE ring,
            # in parallel with chunk 0 on SWDGE and weights on the SP ring.
            scalar.dma_start(
                xt_sb[:, 0, 0:2, SC : 2 * SC], xt_view[:, 0:2, SC : 2 * SC]
            ).then_inc(sem_xt2A, 16)
            scalar.dma_start(
                xt_sb[:, 0, 2:4, SC : 2 * SC], xt_view[:, 2:4, SC : 2 * SC]
            ).then_inc(sem_xt2B, 16)
            scalar.dma_start(
                xt_sb[:, 0, KT // 2 :, SC : 2 * SC],
                xt_view[:, KT // 2 :, SC : 2 * SC],
            ).then_inc(sem_xt2, 16)
            if mode == "pe":
                return
            scalar.wait_ge(sem_b, 16)  # biases resident
            for u in range(nsc):
                ub = u % 2
                for j in range(JB):  # sig(g), both chunks
                    if u >= 2:
                        # this parity's sig_g slot j was read by DVE mults
                        # two superchunks back
                        scalar.wait_ge(sem_dve, dve_mult(u - 2, 1, j))
                    scalar.wait_ge(sem_pe, grp_done(u, 0, j, 1))
                    scalar.activation(
                        sig_g[:, ub, :, j, :], ps[j][:, :, :], AF.Sigmoid,
                        bias=bias_sb[:, 3 * j : 3 * j + 1],
                    ).then_inc(sem_act, 1)
                for j in range(JB):  # tanh(v), both chunks
                    scalar.wait_ge(sem_pe, grp_done(u, 1, j, 1))
                    scalar.activation(
                        tanh_v[:, ub, :, j, :], ps[j][:, :, :], AF.Tanh,
                        bias=bias_sb[:, 3 * j + 1 : 3 * j + 2],
                    ).then_inc(sem_act, 1)
                for j in range(JB):  # sig(d), per (j, t) so the tail is fine
                    for t in range(2):
                        if u >= 2:
                            scalar.wait_ge(sem_dve, dve_aff(u - 2, t, j))
                        scalar.wait_ge(sem_pe, grp_done(u, 2, j, t))
                        scalar.activation(
                            sig_d[:, ub, t, j, :], ps[j][:, t, :], AF.Sigmoid,
                            bias=bias_sb[:, 3 * j + 2 : 3 * j + 3],
                        ).then_inc(sem_act, 1)

        @block.vector
        def _(vector):
            if N_WARMUP:
                vector.memset(warm_sb[:], 1.0).then_inc(sem_warm, 1)
            if mode != "full":
                return
            for u in range(nsc):
                ub = u % 2
                for j in range(JB):
                    for t in range(2):
                        c = 2 * u + t
                        vector.wait_ge(sem_act, act_v(u, j))
                        if u >= 1:
                            # own-engine WAR: xs_t slot was read by last
                            # superchunk's scan
                            vector.wait_ge(sem_dve, dve_scan(u - 1, t, j))
                        vector.tensor_tensor(
                            xs_t[:, t, j, :], sig_g[:, ub, t, j, :],
                            tanh_v[:, ub, t, j, :], OP.mult,
                        ).then_inc(sem_dve, 1)
                        # decay affine: a = 0.998 * sigmoid(d) + 0.001
                        vector.wait_ge(sem_act, act_d(u, t, j))
                        vector.tensor_scalar(
                            a_t[:, t, j, :], sig_d[:, ub, t, j, :], 0.998, 0.001,
                            op0=OP.mult, op1=OP.add,
                        ).then_inc(sem_dve, 1)
                        if c >= 2:
                            # h slot (c%2, j) was read by store (c-2, j)
                            pp = st_pos(c - 2, j)
                            vector.wait_ge(st_sem(pp), st_val(pp))
                        # own-engine RAW on a_t/xs_t: the race detector wants
                        # an explicit sync; satisfied at issue (in-order DVE)
                        vector.wait_ge(sem_dve, dve_aff(u, t, j))
                        init = (
                            0.0 if c == 0
                            else h_t[:, (c - 1) % 2, j, SC - 1 : SC]
                        )
                        vector.tensor_tensor_scan(
                            h_t[:, c % 2, j, :], a_t[:, t, j, :],
                            xs_t[:, t, j, :], init, OP.mult, OP.add,
                        ).then_inc(sem_dve, 1)

        @block.sync
        def _(sync):
            # weights/biases ride the otherwise-idle SP HWDGE ring at startup,
            # overlapping the chunk loads on the SWDGE + ACT rings
            # weights first — bias is only needed by ACT ~6us in, and the
            # first Wg quarter is on the PE's time-to-first-matmul path
            sync.dma_start(
                w_sb[:, 0, 0:2, :], w_view[:, 0, 0:2, :]
            ).then_inc(sem_wA, 16)
            sync.dma_start(
                w_sb[:, 0, 2:4, :], w_view[:, 0, 2:4, :]
            ).then_inc(sem_wB, 16)
            sync.dma_start(
                w_sb[:, 0, KT // 2 :, :], w_view[:, 0, KT // 2 :, :]
            ).then_inc(sem_w, 16)
            sync.dma_start(bias_sb[:], bias_d).then_inc(sem_b, 16)
            sync.wait_ge(sem_pe, 2)
            sync.dma_start(w_sb[:, 1, :, :], w_view[:, 1, :, :]).then_inc(sem_w, 16)
            sync.wait_ge(sem_pe, 10)
            sync.dma_start(w_sb[:, 2, :, :], w_view[:, 2, :, :]).then_inc(sem_w, 16)
            if mode != "full":
                return
            for u in range(nch // 2):
                for j in range(JB):
                    for t in range(2):
                        c = 2 * u + t
                        p = st_pos(c, j)
                        if p >= 3:
                            # keep each sem's increments strictly sequential
                            # (chain on the previous store of the SAME sem,
                            # two stores back — long completed)
                            sync.wait_ge(st_sem(p - 2), st_val(p - 2))
                        sync.wait_ge(sem_dve, dve_scan(u, t, j))
                        sync.dma_start(
                            ht_view[:, j, SC * (c % NCH) : SC * (c % NCH + 1)],
                            h_t[:, c % 2, j, :],
                        ).then_inc(st_sem(p), 16)

    return nc


_NC_CACHE = None


def _build_in_maps(inputs):
    x = np.asarray(inputs["x"], dtype=np.float32)
    Wg = np.asarray(inputs["Wg"], dtype=np.float32)
    bg = np.asarray(inputs["bg"], dtype=np.float32)
    Wv = np.asarray(inputs["Wv"], dtype=np.float32)
    bv = np.asarray(inputs["bv"], dtype=np.float32)
    Wd = np.asarray(inputs["Wd"], dtype=np.float32)
    bd = np.asarray(inputs["bd"], dtype=np.float32)

    in_maps = []
    for core in range(8):
        b, eh = divmod(core, 2)
        sl = slice(E * eh, E * (eh + 1))
        xt = x[b].T.astype(np.float16)                          # (D, S)
        w = np.stack([Wg[:, sl], Wv[:, sl], Wd[:, sl]], axis=0).astype(
            np.float16
        )                                                       # (3, D, E)
        bias = np.empty((128, 3 * JB), dtype=np.float32)
        for pi, barr in enumerate((bg[sl], bv[sl], bd[sl])):
            b4 = barr.reshape(JB, 128)
            for j in range(JB):
                bias[:, 3 * j + pi] = b4[j]
        in_maps.append({"xt": xt, "w": w, "bias": bias})
    return in_maps


def kernel(**inputs: np.ndarray) -> np.ndarray:
    global _NC_CACHE
    if _NC_CACHE is None:
        _NC_CACHE = _build_bass()
    nc = _NC_CACHE

    in_maps = _build_in_maps(inputs)
    res = bass_utils.run_bass_kernel_spmd(nc, in_maps, core_ids=list(range(8)))

    out = np.empty((B, S, D), dtype=np.float32)
    for core in range(8):
        b, eh = divmod(core, 2)
        out[b, :, E * eh : E * (eh + 1)] = res.results[core]["ht"].astype(
            np.float32
        ).T
    return out


# revision 58
# speedup vs baseline: 1.0019x; 1.0019x over previous
"""MinGRU layer Trainium2 kernel.

Math (per batch b):
    g = x @ Wg + bg ; v = x @ Wv + bv ; d = x @ Wd + bd
    xs = sigmoid(g) * tanh(v) ; a = 0.001 + 0.998 * sigmoid(d)
    h_t = a_t * h_{t-1} + xs_t  (h_0 = 0, scan over time S)

Sharding: 8 cores = 4 batches x 2 halves of the 1024 output features.
Each core computes h^T[e, s] for its (b, e-half) with zero cross-core
communication; the time recurrence runs on-chip via the VectorE
TensorTensorScan instruction (time on the free axis, features on
partitions; scan state is fp32 internally regardless of operand dtype).

Precision: matmul inputs (x, W) and all post-activation intermediates
are fp16 (PSUM accumulation and the scan state stay fp32) — measured
end-to-end rel err ~4e-4 vs the 2e-2 budget, and it halves DMA traffic
and SBUF footprint. Host feeds x transposed (d-major) so every device
access is contiguous.

Schedule per superchunk u (= chunk pair 2u, 2u+1):
  PE: warmup matmuls at t=0 ramp the HAM clock gate; then for p(3
      proj): for j(4 e-blocks): k-loop with the two chunks' matmuls
      interleaved on banks (same-bank back-to-back accumulation is
      slow), sharing each weight tile.
  ACT: sig(g) and tanh(v) per (j) over both chunks; sig(d) per (t, j)
      so the tail drains at (t, j) granularity.
  DVE: per (t, j): gating multiply, decay affine a = 0.998*sig(d) +
      0.001, then the scan with carry chaining.
  SP: weights/bias at startup, then one store per (chunk, j).
  POOL/SCALAR: input DMAs on two independent rings at startup.
"""

import os
import sys

for _p in ("/opt/trn_rl_repo", "/root/.axon_site/_ro/trn_rl_repo"):
    if os.path.isdir(_p) and _p not in sys.path:
        sys.path.insert(0, _p)

import numpy as np

import concourse.bass as bass
import concourse.mybir as mybir
from concourse import bass_utils

B, S, D = 4, 4096, 1024
E = 512                # output features per core (D / 2)
NCH = 8                # time chunks
SC = S // NCH          # chunk length (512)
KT = D // 128          # contraction tiles (8)
JB = E // 128          # output-feature blocks per core (4)

F32 = mybir.dt.float32
F16 = mybir.dt.float16
AF = mybir.ActivationFunctionType
OP = mybir.AluOpType

N_WARMUP = 20          # 128-col PE matmuls bridging t=0 to the first real
                       # matmul (~2.8us) so the clock-gate ramp starts early


def _build_bass(nch=NCH, mode="full"):
    """Build the Bass program. nch > NCH replays the 8 data chunks multiple
    times (benchmarking only — amortizes host/RPC overhead out of timing).
    mode="pe" keeps only PE + input DMAs (bottleneck isolation)."""
    assert nch % 2 == 0
    nc = bass.Bass("TRN2", target_bir_lowering=False, debug=False, num_devices=8)

    xt_d = nc.dram_tensor("xt", [D, S], F16, kind="ExternalInput").ap()
    w_d = nc.dram_tensor("w", [3, D, E], F16, kind="ExternalInput").ap()
    bias_d = nc.dram_tensor("bias", [128, 3 * JB], F32, kind="ExternalInput").ap()
    ht_d = nc.dram_tensor("ht", [E, S], F16, kind="ExternalOutput").ap()

    from contextlib import ExitStack

    with ExitStack() as ctx:
        block = ctx.enter_context(nc.Block())
        sem_xt = ctx.enter_context(nc.semaphore("sem_xt"))
        sem_xtA = ctx.enter_context(nc.semaphore("sem_xtA"))
        sem_xtB = ctx.enter_context(nc.semaphore("sem_xtB"))
        sem_xt2 = ctx.enter_context(nc.semaphore("sem_xt2"))
        sem_xt2A = ctx.enter_context(nc.semaphore("sem_xt2A"))
        sem_xt2B = ctx.enter_context(nc.semaphore("sem_xt2B"))
        sem_w = ctx.enter_context(nc.semaphore("sem_w"))
        sem_wA = ctx.enter_context(nc.semaphore("sem_wA"))
        sem_wB = ctx.enter_context(nc.semaphore("sem_wB"))
        sem_b = ctx.enter_context(nc.semaphore("sem_b"))
        sem_warm = ctx.enter_context(nc.semaphore("sem_warm"))
        sem_pe = ctx.enter_context(nc.semaphore("sem_pe"))
        sem_act = ctx.enter_context(nc.semaphore("sem_act"))
        sem_dve = ctx.enter_context(nc.semaphore("sem_dve"))
        # stores alternate between two sems so consecutive stores never
        # chain-wait on each other's completion (the ~0.9us DMA-sem
        # propagation would otherwise sit on the drain's critical path)
        sem_st = ctx.enter_context(nc.semaphore("sem_st"))
        sem_st2 = ctx.enter_context(nc.semaphore("sem_st2"))
        w_sb = ctx.enter_context(nc.sbuf_tensor("w_sb", [128, 3, KT, E], F16))
        # two pair-slots: each holds a superchunk (2 chunks side by side on
        # the free axis) so steady-state loads are one DMA with 2KB runs
        xt_sb = ctx.enter_context(
            nc.sbuf_tensor("xt_sb", [128, 2, KT, 2 * SC], F16)
        )
        bias_sb = ctx.enter_context(nc.sbuf_tensor("bias_sb", [128, 3 * JB], F32))
        warm_sb = ctx.enter_context(nc.sbuf_tensor("warm_sb", [128, 128], F16))
        # leading dim: superchunk parity (double buffer) — without it the
        # ACT(u) ops chain on DVE(u-1) ops which chain on ACT(u-1), aligning
        # the whole consumer pipeline just-in-time behind the PE and costing
        # the PE ~426ns at every (p, j) block boundary
        sig_g = ctx.enter_context(nc.sbuf_tensor("sig_g", [128, 2, 2, JB, SC], F16))
        tanh_v = ctx.enter_context(nc.sbuf_tensor("tanh_v", [128, 2, 2, JB, SC], F16))
        sig_d = ctx.enter_context(nc.sbuf_tensor("sig_d", [128, 2, 2, JB, SC], F16))
        a_t = ctx.enter_context(nc.sbuf_tensor("a_t", [128, 2, JB, SC], F16))
        xs_t = ctx.enter_context(nc.sbuf_tensor("xs_t", [128, 2, JB, SC], F16))
        h_t = ctx.enter_context(nc.sbuf_tensor("h_t", [128, 2, JB, SC], F16))
        ps = []
        for j in range(JB):
            ps_j = ctx.enter_context(nc.psum_tensor(f"ps{j}", [128, 2, SC], F32))
            ps.append(ps_j)

        # x^T viewed as [p, k, s]; row index of xt is d = 128*k + p
        xt_view = xt_d.rearrange("(k p) s -> p k s", p=128)
        # weights viewed as [p, proj, k, e]
        w_view = w_d.rearrange("q (k p) e -> p q k e", p=128)
        # h^T viewed as [p, j, s]; row index of ht is e = 128*j + p
        ht_view = ht_d.rearrange("(j p) s -> p j s", p=128)

        nsc = nch // 2

        # PE group counter: groups complete in (u, p, j, t) order
        def grp_done(u, p, j, t):
            return 24 * u + 8 * p + 2 * j + t + 1

        # ACT op counter per superchunk: g(j0..3), v(j0..3), d(t0,j0..3),
        # d(t1,j0..3) = 16 ops
        def act_g(u, j):
            return 16 * u + 1 + j

        def act_v(u, j):
            return 16 * u + 5 + j

        def act_d(u, t, j):
            # d ops run (j outer, t inner) so each j's pair completes right
            # after its PE p2 group — keeps the drain j-granular
            return 16 * u + 9 + 2 * j + t

        # DVE op counter per superchunk: per j: mult(t0), mult(t1), aff(t0),
        # scan(t0), aff(t1), scan(t1). j outer so only j3's groups drain
        # after the PE's final p2 group; both mults lead because they only
        # need ACT g/v (ready early) — keeps them off the drain critical
        # path. Per-j scan carry chains stay in order (t0 before t1).
        def dve_mult(u, t, j):
            return 24 * u + 6 * j + 1 + t

        def dve_aff(u, t, j):
            return 24 * u + 6 * j + 3 + 2 * t

        def dve_scan(u, t, j):
            return 24 * u + 6 * j + 4 + 2 * t

        # store counter: (u, j, t) order matching scan completion order.
        # Store #p (1-based) rides sem_st if p is odd, sem_st2 if even, and
        # is that sem's ((p+1)//2)-th increment.
        def st_pos(c, j):
            return 8 * (c // 2) + 2 * j + (c % 2) + 1

        def st_sem(p):
            return sem_st if p % 2 == 1 else sem_st2

        def st_val(p):
            return 16 * ((p + 1) // 2)

        @block.gpsimd
        def _(gpsimd):
            # Cumulative-sem soundness: SDMA engine-slots drain independently,
            # so a threshold 16*n on a sem is only sound when ALL DMAs queued
            # on that sem at that point are covered by it. Hence separate
            # sems per stream; later loads are queue-gated on sem_pe so every
            # downstream wait is a full-prefix wait.
            # Chunk 0 rides here (SWDGE); chunk 1 rides the scalar HWDGE ring
            # in parallel. k-halves let PE start on the low contraction tiles.
            # each startup piece rides its own semaphore: two unguarded DMAs
            # on one sem drain their 16 slot-increments interleaved, so a
            # partial threshold would be unsound. Quarters keep the PE's
            # time-to-first-matmul at ~2.6us (k0-1 arrive first).
            gpsimd.dma_start(
                xt_sb[:, 0, 0:2, 0:SC], xt_view[:, 0:2, 0:SC]
            ).then_inc(sem_xtA, 16)
            gpsimd.dma_start(
                xt_sb[:, 0, 2:4, 0:SC], xt_view[:, 2:4, 0:SC]
            ).then_inc(sem_xtB, 16)
            gpsimd.dma_start(
                xt_sb[:, 0, KT // 2 :, 0:SC], xt_view[:, KT // 2 :, 0:SC]
            ).then_inc(sem_xt, 16)
            for up in range(1, nch // 2):
                # pair up's slot (up%2) was last used by pair up-2, consumed
                # by the end of superchunk up-2 — a full superchunk of
                # prefetch lead. The sem_xt chain wait keeps this sem's
                # increments strictly sequential (DMA slot-completions
                # interleave otherwise). sem_xt counts: c0-hi=16, pair up at
                # 16*(up+1).
                gpsimd.wait_ge(sem_xt, 16 * up)
                if up == 1:
                    # throttle off the startup-critical first microseconds
                    gpsimd.wait_ge(sem_pe, 2)
                else:
                    gpsimd.wait_ge(sem_pe, grp_done(up - 2, 2, 3, 1))
                s_lo = SC * ((2 * up) % NCH)
                gpsimd.dma_start(
                    xt_sb[:, up % 2, :, :],
                    xt_view[:, :, s_lo : s_lo + 2 * SC],
                ).then_inc(sem_xt, 16)

        @block.tensor
        def _(tensor):
            # Warmup: tiny matmuls on a DVE-memset SBUF tile ramp the PE
            # HAM clock gate toward full speed while the first DMAs stream
            # in; their psum garbage is overwritten by the first real
            # start=True group.
            if N_WARMUP:
                tensor.wait_ge(sem_warm, 1)
                for _ in range(N_WARMUP):
                    tensor.matmul(
                        ps[0][0:8, 0, 0:128], warm_sb[:, 0:8], warm_sb[:, :],
                        start=True, stop=True,
                    )
            for u in range(nsc):
                if u >= 1:
                    # this pair resident (pair u lands at 16*(u+1))
                    tensor.wait_ge(sem_xt, 16 * (u + 1))
                sl = u % 2
                for p in range(3):
                    if u == 0 and p >= 1:
                        # this projection's weights resident (p=0 is gated
                        # k-granularly inside the first j-loop below)
                        tensor.wait_ge(sem_w, (32, 48)[p - 1])
                    for j in range(JB):
                        # banks (2j, 2j+1) were written by the previous
                        # p-block; the first matmul of this block carries a
                        # wait for the ACT ops that read them (attached, not
                        # a standalone EventSemaphore — a standalone wait
                        # breaks the PE pipeline and costs ~426ns/block)
                        if (u, p) == (0, 0) or mode == "pe":
                            blk_wait = None
                        elif p == 0:
                            blk_wait = act_d(u - 1, 1, j)
                        elif p == 1:
                            blk_wait = act_g(u, j)
                        else:
                            blk_wait = act_v(u, j)
                        for k in range(KT):
                            if u == 0 and p == 0 and j == 0:
                                if k == 0:
                                    # first quarter (k0-1) of both startup
                                    # chunks and of the Wg weights
                                    tensor.wait_ge(sem_xtA, 16)
                                    tensor.wait_ge(sem_xt2A, 16)
                                    tensor.wait_ge(sem_wA, 16)
                                elif k == 2:
                                    tensor.wait_ge(sem_xtB, 16)
                                    tensor.wait_ge(sem_xt2B, 16)
                                    tensor.wait_ge(sem_wB, 16)
                                elif k == KT // 2:
                                    tensor.wait_ge(sem_xt, 16)
                                    tensor.wait_ge(sem_xt2, 16)
                                    tensor.wait_ge(sem_w, 16)
                            w_ap = w_sb[:, p, k, 128 * j : 128 * (j + 1)]
                            m0 = tensor.matmul(
                                ps[j][:, 0, :], w_ap,
                                xt_sb[:, sl, k, 0:SC],
                                start=(k == 0), stop=(k == KT - 1),
                            )
                            if k == 0 and blk_wait is not None:
                                m0._wait_ge(sem_act, blk_wait)
                            m1 = tensor.matmul(
                                ps[j][:, 1, :], w_ap,
                                xt_sb[:, sl, k, SC : 2 * SC],
                                start=(k == 0), stop=(k == KT - 1),
                            )
                        # per-t incs: odd sem_pe values mean "t0 group done"
                        # (one matmul earlier); even values land exactly when
                        # the old +2 did, so all even thresholds are unchanged
                        m0.then_inc(sem_pe, 1)
                        m1.then_inc(sem_pe, 1)

        @block.scalar
        def _(scalar):
            # Startup: chunk 1 loads ride the otherwise-idle ACT HWDGE ring,
            # in parallel with chunk 0 on SWDGE and weights on the SP ring.
            scalar.dma_start(
                xt_sb[:, 0, 0:2, SC : 2 * SC], xt_view[:, 0:2, SC : 2 * SC]
            ).then_inc(sem_xt2A, 16)
            scalar.dma_start(
                xt_sb[:, 0, 2:4, SC : 2 * SC], xt_view[:, 2:4, SC : 2 * SC]
            ).then_inc(sem_xt2B, 16)
            scalar.dma_start(
                xt_sb[:, 0, KT // 2 :, SC : 2 * SC],
                xt_view[:, KT // 2 :, SC : 2 * SC],
            ).then_inc(sem_xt2, 16)
            if mode == "pe":
                return
            scalar.wait_ge(sem_b, 16)  # biases resident
            for u in range(nsc):
                ub = u % 2
                for j in range(JB):  # sig(g), both chunks
                    if u >= 2:
                        # this parity's sig_g slot j was read by DVE mults
                        # two superchunks back
                        scalar.wait_ge(sem_dve, dve_mult(u - 2, 1, j))
                    scalar.wait_ge(sem_pe, grp_done(u, 0, j, 1))
                    scalar.activation(
                        sig_g[:, ub, :, j, :], ps[j][:, :, :], AF.Sigmoid,
                        bias=bias_sb[:, 3 * j : 3 * j + 1],
                    ).then_inc(sem_act, 1)
                for j in range(JB):  # tanh(v), both chunks
                    scalar.wait_ge(sem_pe, grp_done(u, 1, j, 1))
                    scalar.activation(
                        tanh_v[:, ub, :, j, :], ps[j][:, :, :], AF.Tanh,
                        bias=bias_sb[:, 3 * j + 1 : 3 * j + 2],
                    ).then_inc(sem_act, 1)
                for j in range(JB):  # sig(d), per (j, t) so the tail is fine
                    for t in range(2):
                        if u >= 2:
                            scalar.wait_ge(sem_dve, dve_aff(u - 2, t, j))
                        scalar.wait_ge(sem_pe, grp_done(u, 2, j, t))
                        scalar.activation(
                            sig_d[:, ub, t, j, :], ps[j][:, t, :], AF.Sigmoid,
                            bias=bias_sb[:, 3 * j + 2 : 3 * j + 3],
                        ).then_inc(sem_act, 1)

        @block.vector
        def _(vector):
            if N_WARMUP:
                vector.memset(warm_sb[:], 1.0).then_inc(sem_warm, 1)
            if mode != "full":
                return
            for u in range(nsc):
                ub = u % 2
                for j in range(JB):
                    for t in range(2):
                        vector.wait_ge(sem_act, act_v(u, j))
                        if u >= 1:
                            # own-engine WAR: xs_t/a_t slots were read by
                            # last superchunk's scans (the t1 wait also
                            # covers the affines that follow)
                            vector.wait_ge(sem_dve, dve_scan(u - 1, t, j))
                        vector.tensor_tensor(
                            xs_t[:, t, j, :], sig_g[:, ub, t, j, :],
                            tanh_v[:, ub, t, j, :], OP.mult,
                        ).then_inc(sem_dve, 1)
                    for t in range(2):
                        c = 2 * u + t
                        # decay affine: a = 0.998 * sigmoid(d) + 0.001
                        vector.wait_ge(sem_act, act_d(u, t, j))
                        vector.tensor_scalar(
                            a_t[:, t, j, :], sig_d[:, ub, t, j, :], 0.998, 0.001,
                            op0=OP.mult, op1=OP.add,
                        ).then_inc(sem_dve, 1)
                        if c >= 2:
                            # h slot (c%2, j) was read by store (c-2, j)
                            pp = st_pos(c - 2, j)
                            vector.wait_ge(st_sem(pp), st_val(pp))
                        # own-engine RAW on a_t/xs_t: the race detector wants
                        # an explicit sync; satisfied at issue (in-order DVE)
                        vector.wait_ge(sem_dve, dve_aff(u, t, j))
                        init = (
                            0.0 if c == 0
                            else h_t[:, (c - 1) % 2, j, SC - 1 : SC]
                        )
                        vector.tensor_tensor_scan(
                            h_t[:, c % 2, j, :], a_t[:, t, j, :],
                            xs_t[:, t, j, :], init, OP.mult, OP.add,
                        ).then_inc(sem_dve, 1)

        @block.sync
        def _(sync):
            # weights/biases ride the otherwise-idle SP HWDGE ring at startup,
            # overlapping the chunk loads on the SWDGE + ACT rings
            # weights first — bias is only needed by ACT ~6us in, and the
            # first Wg quarter is on the PE's time-to-first-matmul path
            sync.dma_start(
                w_sb[:, 0, 0:2, :], w_view[:, 0, 0:2, :]
            ).then_inc(sem_wA, 16)
            sync.dma_start(
                w_sb[:, 0, 2:4, :], w_view[:, 0, 2:4, :]
            ).then_inc(sem_wB, 16)
            sync.dma_start(
                w_sb[:, 0, KT // 2 :, :], w_view[:, 0, KT // 2 :, :]
            ).then_inc(sem_w, 16)
            sync.dma_start(bias_sb[:], bias_d).then_inc(sem_b, 16)
            sync.wait_ge(sem_pe, 2)
            sync.dma_start(w_sb[:, 1, :, :], w_view[:, 1, :, :]).then_inc(sem_w, 16)
            sync.wait_ge(sem_pe, 10)
            sync.dma_start(w_sb[:, 2, :, :], w_view[:, 2, :, :]).then_inc(sem_w, 16)
            if mode != "full":
                return
            for u in range(nch // 2):
                for j in range(JB):
                    for t in range(2):
                        c = 2 * u + t
                        p = st_pos(c, j)
                        if p >= 3:
                            # keep each sem's increments strictly sequential
                            # (chain on the previous store of the SAME sem,
                            # two stores back — long completed)
                            sync.wait_ge(st_sem(p - 2), st_val(p - 2))
                        sync.wait_ge(sem_dve, dve_scan(u, t, j))
                        sync.dma_start(
                            ht_view[:, j, SC * (c % NCH) : SC * (c % NCH + 1)],
                            h_t[:, c % 2, j, :],
                        ).then_inc(st_sem(p), 16)

    return nc


_NC_CACHE = None


def _build_in_maps(inputs):
    x = np.asarray(inputs["x"], dtype=np.float32)
    Wg = np.asarray(inputs["Wg"], dtype=np.float32)
    bg = np.asarray(inputs["bg"], dtype=np.float32)
    Wv = np.asarray(inputs["Wv"], dtype=np.float32)
    bv = np.asarray(inputs["bv"], dtype=np.float32)
    Wd = np.asarray(inputs["Wd"], dtype=np.float32)
    bd = np.asarray(inputs["bd"], dtype=np.float32)

    in_maps = []
    for core in range(8):
        b, eh = divmod(core, 2)
        sl = slice(E * eh, E * (eh + 1))
        xt = x[b].T.astype(np.float16)                          # (D, S)
        w = np.stack([Wg[:, sl], Wv[:, sl], Wd[:, sl]], axis=0).astype(
            np.float16
        )                                                       # (3, D, E)
        bias = np.empty((128, 3 * JB), dtype=np.float32)
        for pi, barr in enumerate((bg[sl], bv[sl], bd[sl])):
            b4 = barr.reshape(JB, 128)
            for j in range(JB):
                bias[:, 3 * j + pi] = b4[j]
        in_maps.append({"xt": xt, "w": w, "bias": bias})
    return in_maps


def kernel(**inputs: np.ndarray) -> np.ndarray:
    global _NC_CACHE
    if _NC_CACHE is None:
        _NC_CACHE = _build_bass()
    nc = _NC_CACHE

    in_maps = _build_in_maps(inputs)
    res = bass_utils.run_bass_kernel_spmd(nc, in_maps, core_ids=list(range(8)))

    out = np.empty((B, S, D), dtype=np.float32)
    for core in range(8):
        b, eh = divmod(core, 2)
        out[b, :, E * eh : E * (eh + 1)] = res.results[core]["ht"].astype(
            np.float32
        ).T
    return out


# revision 64
# speedup vs baseline: 1.0025x; 1.0006x over previous
"""MinGRU layer Trainium2 kernel.

Math (per batch b):
    g = x @ Wg + bg ; v = x @ Wv + bv ; d = x @ Wd + bd
    xs = sigmoid(g) * tanh(v) ; a = 0.001 + 0.998 * sigmoid(d)
    h_t = a_t * h_{t-1} + xs_t  (h_0 = 0, scan over time S)

Sharding: 8 cores = 4 batches x 2 halves of the 1024 output features.
Each core computes h^T[e, s] for its (b, e-half) with zero cross-core
communication; the time recurrence runs on-chip via the VectorE
TensorTensorScan instruction (time on the free axis, features on
partitions; scan state is fp32 internally regardless of operand dtype).

Precision: matmul inputs (x, W) and all post-activation intermediates
are fp16 (PSUM accumulation and the scan state stay fp32) — measured
end-to-end rel err ~4e-4 vs the 2e-2 budget, and it halves DMA traffic
and SBUF footprint. Host feeds x transposed (d-major) so every device
access is contiguous.

Schedule per superchunk u (= chunk pair 2u, 2u+1):
  PE: warmup matmuls at t=0 ramp the HAM clock gate; then for p(3
      proj): for j(4 e-blocks): k-loop with the two chunks' matmuls
      interleaved on banks (same-bank back-to-back accumulation is
      slow), sharing each weight tile.
  ACT: sig(g) and tanh(v) per (j) over both chunks; sig(d) per (t, j)
      so the tail drains at (t, j) granularity.
  DVE: per (t, j): gating multiply, decay affine a = 0.998*sig(d) +
      0.001, then the scan with carry chaining.
  SP: weights/bias at startup, then one store per (chunk, j).
  POOL/SCALAR: input DMAs on two independent rings at startup.
"""

import os
import sys

for _p in ("/opt/trn_rl_repo", "/root/.axon_site/_ro/trn_rl_repo"):
    if os.path.isdir(_p) and _p not in sys.path:
        sys.path.insert(0, _p)

import numpy as np

import concourse.bass as bass
import concourse.mybir as mybir
from concourse import bass_utils

B, S, D = 4, 4096, 1024
E = 512                # output features per core (D / 2)
NCH = 8                # time chunks
SC = S // NCH          # chunk length (512)
KT = D // 128          # contraction tiles (8)
JB = E // 128          # output-feature blocks per core (4)

F32 = mybir.dt.float32
F16 = mybir.dt.float16
AF = mybir.ActivationFunctionType
OP = mybir.AluOpType

N_WARMUP = 15          # 128-col PE matmuls bridging t=0 to the first real
                       # matmul (~2.2us) so the clock-gate ramp starts early


def _build_bass(nch=NCH, mode="full"):
    """Build the Bass program. nch > NCH replays the 8 data chunks multiple
    times (benchmarking only — amortizes host/RPC overhead out of timing).
    mode="pe" keeps only PE + input DMAs (bottleneck isolation)."""
    assert nch % 2 == 0
    nc = bass.Bass("TRN2", target_bir_lowering=False, debug=False, num_devices=8)

    xt_d = nc.dram_tensor("xt", [D, S], F16, kind="ExternalInput").ap()
    w_d = nc.dram_tensor("w", [3, D, E], F16, kind="ExternalInput").ap()
    bias_d = nc.dram_tensor("bias", [128, 3 * JB], F32, kind="ExternalInput").ap()
    ht_d = nc.dram_tensor("ht", [E, S], F16, kind="ExternalOutput").ap()

    from contextlib import ExitStack

    with ExitStack() as ctx:
        block = ctx.enter_context(nc.Block())
        sem_xt = ctx.enter_context(nc.semaphore("sem_xt"))
        sem_xtA = ctx.enter_context(nc.semaphore("sem_xtA"))
        sem_xtB = ctx.enter_context(nc.semaphore("sem_xtB"))
        sem_xtC = ctx.enter_context(nc.semaphore("sem_xtC"))
        sem_xt2 = ctx.enter_context(nc.semaphore("sem_xt2"))
        sem_xt2A = ctx.enter_context(nc.semaphore("sem_xt2A"))
        sem_xt2B = ctx.enter_context(nc.semaphore("sem_xt2B"))
        sem_xt2C = ctx.enter_context(nc.semaphore("sem_xt2C"))
        sem_w = ctx.enter_context(nc.semaphore("sem_w"))
        sem_wA = ctx.enter_context(nc.semaphore("sem_wA"))
        sem_wB = ctx.enter_context(nc.semaphore("sem_wB"))
        sem_wC = ctx.enter_context(nc.semaphore("sem_wC"))
        sem_b = ctx.enter_context(nc.semaphore("sem_b"))
        sem_warm = ctx.enter_context(nc.semaphore("sem_warm"))
        sem_pe = ctx.enter_context(nc.semaphore("sem_pe"))
        sem_act = ctx.enter_context(nc.semaphore("sem_act"))
        sem_dve = ctx.enter_context(nc.semaphore("sem_dve"))
        # stores alternate between two sems so consecutive stores never
        # chain-wait on each other's completion (the ~0.9us DMA-sem
        # propagation would otherwise sit on the drain's critical path)
        sem_st = ctx.enter_context(nc.semaphore("sem_st"))
        sem_st2 = ctx.enter_context(nc.semaphore("sem_st2"))
        w_sb = ctx.enter_context(nc.sbuf_tensor("w_sb", [128, 3, KT, E], F16))
        # two pair-slots: each holds a superchunk (2 chunks side by side on
        # the free axis) so steady-state loads are one DMA with 2KB runs
        xt_sb = ctx.enter_context(
            nc.sbuf_tensor("xt_sb", [128, 2, KT, 2 * SC], F16)
        )
        bias_sb = ctx.enter_context(nc.sbuf_tensor("bias_sb", [128, 3 * JB], F32))
        warm_sb = ctx.enter_context(nc.sbuf_tensor("warm_sb", [128, 128], F16))
        # leading dim: superchunk parity (double buffer) — without it the
        # ACT(u) ops chain on DVE(u-1) ops which chain on ACT(u-1), aligning
        # the whole consumer pipeline just-in-time behind the PE and costing
        # the PE ~426ns at every (p, j) block boundary
        sig_g = ctx.enter_context(nc.sbuf_tensor("sig_g", [128, 2, 2, JB, SC], F16))
        tanh_v = ctx.enter_context(nc.sbuf_tensor("tanh_v", [128, 2, 2, JB, SC], F16))
        sig_d = ctx.enter_context(nc.sbuf_tensor("sig_d", [128, 2, 2, JB, SC], F16))
        a_t = ctx.enter_context(nc.sbuf_tensor("a_t", [128, 2, JB, SC], F16))
        xs_t = ctx.enter_context(nc.sbuf_tensor("xs_t", [128, 2, JB, SC], F16))
        h_t = ctx.enter_context(nc.sbuf_tensor("h_t", [128, 2, JB, SC], F16))
        ps = []
        for j in range(JB):
            ps_j = ctx.enter_context(nc.psum_tensor(f"ps{j}", [128, 2, SC], F32))
            ps.append(ps_j)

        # x^T viewed as [p, k, s]; row index of xt is d = 128*k + p
        xt_view = xt_d.rearrange("(k p) s -> p k s", p=128)
        # weights viewed as [p, proj, k, e]
        w_view = w_d.rearrange("q (k p) e -> p q k e", p=128)
        # h^T viewed as [p, j, s]; row index of ht is e = 128*j + p
        ht_view = ht_d.rearrange("(j p) s -> p j s", p=128)

        nsc = nch // 2

        # PE group counter: groups complete in (u, p, j, t) order
        def grp_done(u, p, j, t):
            return 24 * u + 8 * p + 2 * j + t + 1

        # ACT op counter per superchunk: g(j0..3), v(j0..3), d(t0,j0..3),
        # d(t1,j0..3) = 16 ops
        def act_g(u, j):
            return 16 * u + 1 + j

        def act_v(u, j):
            return 16 * u + 5 + j

        def act_d(u, t, j):
            # d ops run (j outer, t inner) so each j's pair completes right
            # after its PE p2 group — keeps the drain j-granular
            return 16 * u + 9 + 2 * j + t

        # DVE op counter per superchunk: per j: mult(t0), mult(t1), aff(t0),
        # scan(t0), aff(t1), scan(t1). j outer so only j3's groups drain
        # after the PE's final p2 group; both mults lead because they only
        # need ACT g/v (ready early) — keeps them off the drain critical
        # path. Per-j scan carry chains stay in order (t0 before t1).
        def dve_mult(u, t, j):
            return 24 * u + 6 * j + 1 + t

        def dve_aff(u, t, j):
            return 24 * u + 6 * j + 3 + 2 * t

        def dve_scan(u, t, j):
            return 24 * u + 6 * j + 4 + 2 * t

        # store counter: (u, j, t) order matching scan completion order.
        # Store #p (1-based) rides sem_st if p is odd, sem_st2 if even, and
        # is that sem's ((p+1)//2)-th increment.
        def st_pos(c, j):
            return 8 * (c // 2) + 2 * j + (c % 2) + 1

        def st_sem(p):
            return sem_st if p % 2 == 1 else sem_st2

        def st_val(p):
            return 16 * ((p + 1) // 2)

        @block.gpsimd
        def _(gpsimd):
            # Cumulative-sem soundness: SDMA engine-slots drain independently,
            # so a threshold 16*n on a sem is only sound when ALL DMAs queued
            # on that sem at that point are covered by it. Hence separate
            # sems per stream; later loads are queue-gated on sem_pe so every
            # downstream wait is a full-prefix wait.
            # Chunk 0 rides here (SWDGE); chunk 1 rides the scalar HWDGE ring
            # in parallel. k-halves let PE start on the low contraction tiles.
            # each startup piece rides its own semaphore: two unguarded DMAs
            # on one sem drain their 16 slot-increments interleaved, so a
            # partial threshold would be unsound. Quarters keep the PE's
            # time-to-first-matmul at ~2.6us (k0-1 arrive first).
            gpsimd.dma_start(
                xt_sb[:, 0, 0:1, 0:SC], xt_view[:, 0:1, 0:SC]
            ).then_inc(sem_xtA, 16)
            gpsimd.dma_start(
                xt_sb[:, 0, 1:2, 0:SC], xt_view[:, 1:2, 0:SC]
            ).then_inc(sem_xtB, 16)
            gpsimd.dma_start(
                xt_sb[:, 0, 2:4, 0:SC], xt_view[:, 2:4, 0:SC]
            ).then_inc(sem_xtC, 16)
            gpsimd.dma_start(
                xt_sb[:, 0, KT // 2 :, 0:SC], xt_view[:, KT // 2 :, 0:SC]
            ).then_inc(sem_xt, 16)
            for up in range(1, nch // 2):
                # pair up's slot (up%2) was last used by pair up-2, consumed
                # by the end of superchunk up-2 — a full superchunk of
                # prefetch lead. The sem_xt chain wait keeps this sem's
                # increments strictly sequential (DMA slot-completions
                # interleave otherwise). sem_xt counts: c0-hi=16, pair up at
                # 16*(up+1).
                gpsimd.wait_ge(sem_xt, 16 * up)
                if up == 1:
                    # throttle off the startup-critical first microseconds
                    gpsimd.wait_ge(sem_pe, 2)
                else:
                    gpsimd.wait_ge(sem_pe, grp_done(up - 2, 2, 3, 1))
                s_lo = SC * ((2 * up) % NCH)
                gpsimd.dma_start(
                    xt_sb[:, up % 2, :, :],
                    xt_view[:, :, s_lo : s_lo + 2 * SC],
                ).then_inc(sem_xt, 16)

        @block.tensor
        def _(tensor):
            # Warmup: tiny matmuls on a DVE-memset SBUF tile ramp the PE
            # HAM clock gate toward full speed while the first DMAs stream
            # in; their psum garbage is overwritten by the first real
            # start=True group.
            if N_WARMUP:
                tensor.wait_ge(sem_warm, 1)
                for _ in range(N_WARMUP):
                    tensor.matmul(
                        ps[0][0:8, 0, 0:128], warm_sb[:, 0:8], warm_sb[:, :],
                        start=True, stop=True,
                    )
            for u in range(nsc):
                if u >= 1:
                    # this pair resident (pair u lands at 16*(u+1))
                    tensor.wait_ge(sem_xt, 16 * (u + 1))
                sl = u % 2
                for p in range(3):
                    if u == 0 and p >= 1:
                        # this projection's weights resident (p=0 is gated
                        # k-granularly inside the first j-loop below)
                        tensor.wait_ge(sem_w, (32, 48)[p - 1])
                    for j in range(JB):
                        # banks (2j, 2j+1) were written by the previous
                        # p-block; the first matmul of this block carries a
                        # wait for the ACT ops that read them (attached, not
                        # a standalone EventSemaphore — a standalone wait
                        # breaks the PE pipeline and costs ~426ns/block)
                        if (u, p) == (0, 0) or mode == "pe":
                            blk_wait = None
                        elif p == 0:
                            blk_wait = act_d(u - 1, 1, j)
                        elif p == 1:
                            blk_wait = act_g(u, j)
                        else:
                            blk_wait = act_v(u, j)
                        for k in range(KT):
                            if u == 0 and p == 0 and j == 0:
                                # k-granular startup gating: k0, k1, k2-3,
                                # k4-7 arrive as separate pieces per stream
                                if k == 0:
                                    tensor.wait_ge(sem_xtA, 16)
                                    tensor.wait_ge(sem_xt2A, 16)
                                    tensor.wait_ge(sem_wA, 16)
                                elif k == 1:
                                    tensor.wait_ge(sem_xtB, 16)
                                    tensor.wait_ge(sem_xt2B, 16)
                                    tensor.wait_ge(sem_wB, 16)
                                elif k == 2:
                                    tensor.wait_ge(sem_xtC, 16)
                                    tensor.wait_ge(sem_xt2C, 16)
                                    tensor.wait_ge(sem_wC, 16)
                                elif k == KT // 2:
                                    tensor.wait_ge(sem_xt, 16)
                                    tensor.wait_ge(sem_xt2, 16)
                                    tensor.wait_ge(sem_w, 16)
                            w_ap = w_sb[:, p, k, 128 * j : 128 * (j + 1)]
                            m0 = tensor.matmul(
                                ps[j][:, 0, :], w_ap,
                                xt_sb[:, sl, k, 0:SC],
                                start=(k == 0), stop=(k == KT - 1),
                            )
                            if k == 0 and blk_wait is not None:
                                m0._wait_ge(sem_act, blk_wait)
                            m1 = tensor.matmul(
                                ps[j][:, 1, :], w_ap,
                                xt_sb[:, sl, k, SC : 2 * SC],
                                start=(k == 0), stop=(k == KT - 1),
                            )
                        # per-t incs: odd sem_pe values mean "t0 group done"
                        # (one matmul earlier); even values land exactly when
                        # the old +2 did, so all even thresholds are unchanged
                        m0.then_inc(sem_pe, 1)
                        m1.then_inc(sem_pe, 1)

        @block.scalar
        def _(scalar):
            # Startup: chunk 1 loads ride the otherwise-idle ACT HWDGE ring,
            # in parallel with chunk 0 on SWDGE and weights on the SP ring.
            scalar.dma_start(
                xt_sb[:, 0, 0:1, SC : 2 * SC], xt_view[:, 0:1, SC : 2 * SC]
            ).then_inc(sem_xt2A, 16)
            scalar.dma_start(
                xt_sb[:, 0, 1:2, SC : 2 * SC], xt_view[:, 1:2, SC : 2 * SC]
            ).then_inc(sem_xt2B, 16)
            scalar.dma_start(
                xt_sb[:, 0, 2:4, SC : 2 * SC], xt_view[:, 2:4, SC : 2 * SC]
            ).then_inc(sem_xt2C, 16)
            scalar.dma_start(
                xt_sb[:, 0, KT // 2 :, SC : 2 * SC],
                xt_view[:, KT // 2 :, SC : 2 * SC],
            ).then_inc(sem_xt2, 16)
            if mode == "pe":
                return
            scalar.wait_ge(sem_b, 16)  # biases resident
            for u in range(nsc):
                ub = u % 2
                for j in range(JB):  # sig(g), both chunks
                    if u >= 2:
                        # this parity's sig_g slot j was read by DVE mults
                        # two superchunks back
                        scalar.wait_ge(sem_dve, dve_mult(u - 2, 1, j))
                    scalar.wait_ge(sem_pe, grp_done(u, 0, j, 1))
                    scalar.activation(
                        sig_g[:, ub, :, j, :], ps[j][:, :, :], AF.Sigmoid,
                        bias=bias_sb[:, 3 * j : 3 * j + 1],
                    ).then_inc(sem_act, 1)
                for j in range(JB):  # tanh(v), both chunks
                    scalar.wait_ge(sem_pe, grp_done(u, 1, j, 1))
                    scalar.activation(
                        tanh_v[:, ub, :, j, :], ps[j][:, :, :], AF.Tanh,
                        bias=bias_sb[:, 3 * j + 1 : 3 * j + 2],
                    ).then_inc(sem_act, 1)
                for j in range(JB):  # sig(d), per (j, t) so the tail is fine
                    for t in range(2):
                        if u >= 2:
                            scalar.wait_ge(sem_dve, dve_aff(u - 2, t, j))
                        scalar.wait_ge(sem_pe, grp_done(u, 2, j, t))
                        scalar.activation(
                            sig_d[:, ub, t, j, :], ps[j][:, t, :], AF.Sigmoid,
                            bias=bias_sb[:, 3 * j + 2 : 3 * j + 3],
                        ).then_inc(sem_act, 1)

        @block.vector
        def _(vector):
            if N_WARMUP:
                vector.memset(warm_sb[:], 1.0).then_inc(sem_warm, 1)
            if mode != "full":
                return
            for u in range(nsc):
                ub = u % 2
                for j in range(JB):
                    for t in range(2):
                        vector.wait_ge(sem_act, act_v(u, j))
                        if u >= 1:
                            # own-engine WAR: xs_t/a_t slots were read by
                            # last superchunk's scans (the t1 wait also
                            # covers the affines that follow)
                            vector.wait_ge(sem_dve, dve_scan(u - 1, t, j))
                        vector.tensor_tensor(
                            xs_t[:, t, j, :], sig_g[:, ub, t, j, :],
                            tanh_v[:, ub, t, j, :], OP.mult,
                        ).then_inc(sem_dve, 1)
                    for t in range(2):
                        c = 2 * u + t
                        # decay affine: a = 0.998 * sigmoid(d) + 0.001
                        vector.wait_ge(sem_act, act_d(u, t, j))
                        vector.tensor_scalar(
                            a_t[:, t, j, :], sig_d[:, ub, t, j, :], 0.998, 0.001,
                            op0=OP.mult, op1=OP.add,
                        ).then_inc(sem_dve, 1)
                        if c >= 2:
                            # h slot (c%2, j) was read by store (c-2, j)
                            pp = st_pos(c - 2, j)
                            vector.wait_ge(st_sem(pp), st_val(pp))
                        # own-engine RAW on a_t/xs_t: the race detector wants
                        # an explicit sync; satisfied at issue (in-order DVE)
                        vector.wait_ge(sem_dve, dve_aff(u, t, j))
                        init = (
                            0.0 if c == 0
                            else h_t[:, (c - 1) % 2, j, SC - 1 : SC]
                        )
                        vector.tensor_tensor_scan(
                            h_t[:, c % 2, j, :], a_t[:, t, j, :],
                            xs_t[:, t, j, :], init, OP.mult, OP.add,
                        ).then_inc(sem_dve, 1)

        @block.sync
        def _(sync):
            # weights/biases ride the otherwise-idle SP HWDGE ring at startup,
            # overlapping the chunk loads on the SWDGE + ACT rings
            # weights first — bias is only needed by ACT ~6us in, and the
            # first Wg quarter is on the PE's time-to-first-matmul path
            sync.dma_start(
                w_sb[:, 0, 0:1, :], w_view[:, 0, 0:1, :]
            ).then_inc(sem_wA, 16)
            sync.dma_start(
                w_sb[:, 0, 1:2, :], w_view[:, 0, 1:2, :]
            ).then_inc(sem_wB, 16)
            sync.dma_start(
                w_sb[:, 0, 2:4, :], w_view[:, 0, 2:4, :]
            ).then_inc(sem_wC, 16)
            sync.dma_start(
                w_sb[:, 0, KT // 2 :, :], w_view[:, 0, KT // 2 :, :]
            ).then_inc(sem_w, 16)
            sync.dma_start(bias_sb[:], bias_d).then_inc(sem_b, 16)
            sync.wait_ge(sem_pe, 2)
            sync.dma_start(w_sb[:, 1, :, :], w_view[:, 1, :, :]).then_inc(sem_w, 16)
            sync.wait_ge(sem_pe, 10)
            sync.dma_start(w_sb[:, 2, :, :], w_view[:, 2, :, :]).then_inc(sem_w, 16)
            if mode != "full":
                return
            for u in range(nch // 2):
                for j in range(JB):
                    for t in range(2):
                        c = 2 * u + t
                        p = st_pos(c, j)
                        if p >= 3:
                            # keep each sem's increments strictly sequential
                            # (chain on the previous store of the SAME sem,
                            # two stores back — long completed)
                            sync.wait_ge(st_sem(p - 2), st_val(p - 2))
                        sync.wait_ge(sem_dve, dve_scan(u, t, j))
                        sync.dma_start(
                            ht_view[:, j, SC * (c % NCH) : SC * (c % NCH + 1)],
                            h_t[:, c % 2, j, :],
                        ).then_inc(st_sem(p), 16)

    return nc


_NC_CACHE = None


def _build_in_maps(inputs):
    x = np.asarray(inputs["x"], dtype=np.float32)
    Wg = np.asarray(inputs["Wg"], dtype=np.float32)
    bg = np.asarray(inputs["bg"], dtype=np.float32)
    Wv = np.asarray(inputs["Wv"], dtype=np.float32)
    bv = np.asarray(inputs["bv"], dtype=np.float32)
    Wd = np.asarray(inputs["Wd"], dtype=np.float32)
    bd = np.asarray(inputs["bd"], dtype=np.float32)

    in_maps = []
    for core in range(8):
        b, eh = divmod(core, 2)
        sl = slice(E * eh, E * (eh + 1))
        xt = x[b].T.astype(np.float16)                          # (D, S)
        w = np.stack([Wg[:, sl], Wv[:, sl], Wd[:, sl]], axis=0).astype(
            np.float16
        )                                                       # (3, D, E)
        bias = np.empty((128, 3 * JB), dtype=np.float32)
        for pi, barr in enumerate((bg[sl], bv[sl], bd[sl])):
            b4 = barr.reshape(JB, 128)
            for j in range(JB):
                bias[:, 3 * j + pi] = b4[j]
        in_maps.append({"xt": xt, "w": w, "bias": bias})
    return in_maps


def kernel(**inputs: np.ndarray) -> np.ndarray:
    global _NC_CACHE
    if _NC_CACHE is None:
        _NC_CACHE = _build_bass()
    nc = _NC_CACHE

    in_maps = _build_in_maps(inputs)
    res = bass_utils.run_bass_kernel_spmd(nc, in_maps, core_ids=list(range(8)))

    out = np.empty((B, S, D), dtype=np.float32)
    for core in range(8):
        b, eh = divmod(core, 2)
        out[b, :, E * eh : E * (eh + 1)] = res.results[core]["ht"].astype(
            np.float32
        ).T
    return out
